# revision 13
# baseline (speedup 1.0000x reference)
"""Trainium2 Bass kernel for nn_MultiHeadAttention (x:[2,2048,512], 8 heads, d=64).

Sharding: 8 cores = 2 batches x 4 head-pairs. Each core computes the QKV
projection for its 2 heads, the attention, and a partial (row-split) O
projection. Host sums the 4 partials per batch and adds the output bias.

v2 vs baseline (206972ns -> target <80000ns):
 - exp split between ScalarE (exact, gamma-biased) and a custom 7-op DVE
   instruction (EXP_FAST_ANT): bf16-bit-trick exp with a factored quadratic
   octave correction (max rel err ~0.46%, global gain absorbed by the ACT
   bias so softmax cancels it). Removes ACT as the serial bottleneck.
 - the exp scale (128*log2e*SCALE) is folded into wq on the host so the
   score matmul emits DVE-ready pre-scaled logits.
 - x DMA in 4 column chunks with QKV projection + qt0 attention emitted
   per-chunk (load fully overlapped with compute).
 - normalize fused: Z = av_psum * bcast(1/sums) directly (no staging copy).
 - O-projection psum reuses the freed av accumulation banks (8-bank fit
   with double-buffered score and AV psum).
"""

import sys

import numpy as np

for _p in ("/opt/trn_rl_repo",):
    if _p not in sys.path:
        sys.path.insert(0, _p)

import concourse.bass as bass  # noqa: E402
import concourse.tile as tile  # noqa: E402
from concourse import bacc, mybir  # noqa: E402
from concourse.bass_utils import run_bass_kernel_spmd  # noqa: E402

EMBED = 512
NH = 8
HD = 64
S = 2048
B = 2
SCALE = HD ** -0.5
F32 = mybir.dt.float32
BF16 = mybir.dt.bfloat16
I16 = mybir.dt.int16
MM_DT = mybir.dt.float32r

N_KT = EMBED // 128   # 4 contraction k-tiles for the projections
N_QT = S // 512       # 4 q column tiles
N_ST = S // 128       # 16 seq tiles of 128

# ---- fast-exp constants (see module docstring; calibrated offline) ----
# pipeline: w = sp + FE_B; n = w & FE_MASK (floor-to-128 in bf16-bit units);
# g = w - n; out = n + (g*FE_C2)*(g + FE_CA); write-cast int16; bitcast bf16.
# Requires sp = raw_score * FE_A (folded into wq host-side).
FE_A = float(np.float32(128 * np.log2(np.e) * SCALE))
FE_B = 17923.942462614166
FE_C2 = 0.0025459323352996915
FE_CA = 261.89949378833876
FE_LG = 9.031370392726972          # ln(gamma): global gain of the fast path
FE_MASK = 32640.0                  # bits 0x46FF0000: keeps exp=141 + top-7 mantissa
ACT_KAPPA = float(SCALE / FE_A)    # ACT path: exp(kappa*sp + FE_LG) = gamma*exp(SCALE*s)

# which (qt, ks) chunks run exp on the DVE custom op (rest on ScalarE).
# qt0 overlaps the QKV phase where the DVE is busy staging -> lighter share.
def _use_dve(qt: int, ks: int) -> bool:
    # strict alternation (no same-engine runs) once the DVE is free of the
    # qt0 staging work; lighter DVE share during qt0.
    if qt == 0:
        return ks % 4 == 3
    return ks % 2 == 1


# ---- custom DVE op registration (idempotent) ----
def _register_exp_fast():
    import concourse.dve_ops as dvo
    from concourse.dve_spec import AluOp, Bin, C0, C1, C2, Spec, Src0, Src1, lower
    from concourse.dve_uop import DveOpSpec

    if "EXP_FAST_ANT" in dvo._SUB_OPCODE_FOR_NAME:
        return next(op for op in dvo.OPS if op.name == "EXP_FAST_ANT")

    # Src1 (= FE_B, a broadcast const tile) must be consumed at stage 1 —
    # late-stage Src1 reads hard-fault the DVE. C0 carries the late Ca.
    _w = Src0 + Src1
    _n = Bin(AluOp.BITWISE_AND, _w, C1)
    _g = _w - _n
    _body = _n + (_g * C2) * (_g + C0)

    def _ref(in0, in1, c0, c1, c2):
        f32 = np.float32
        w = (np.asarray(in0, f32) + np.asarray(in1, f32)).astype(f32)
        mask = np.asarray(np.asarray(c1, f32)).view(np.uint32)
        n = (w.view(np.uint32) & mask).view(f32)
        g = (w - n).astype(f32)
        z = ((g * f32(c2)).astype(f32) * (g + f32(c0)).astype(f32)).astype(f32)
        return (n + z).astype(f32)

    spec = Spec(body=_body, reference=_ref)
    row = dvo._CUSTOM_DVE_ROW_BASE + len(dvo.OPS)
    assert row < 0x20, "no free custom-DVE opcode row"
    sha = DveOpSpec(
        name="EXP_FAST_ANT", opcode=row, uops=lower(spec, ver="v3"), rd1_en=True
    ).sha("v3")
    op = dvo.DveOp("EXP_FAST_ANT", spec, subdim=False, uops_sha={"v3": sha})
    dvo.OPS.append(op)
    dvo.CUSTOM_DVE_SPECS[op.name] = op.spec
    dvo._SUB_OPCODE_FOR_NAME[op.name] = row
    return op


EXP_FAST_ANT = _register_exp_fast()


def build_nc():
    nc = bacc.Bacc("TRN2", target_bir_lowering=False, debug=False)

    xT_d = nc.dram_tensor("xT", [EMBED, S], MM_DT, kind="ExternalInput").ap()
    wq_d = nc.dram_tensor("wq", [EMBED, 128], MM_DT, kind="ExternalInput").ap()
    wk_d = nc.dram_tensor("wk", [EMBED, 128], MM_DT, kind="ExternalInput").ap()
    wv_d = nc.dram_tensor("wv", [EMBED, 128], MM_DT, kind="ExternalInput").ap()
    wo_d = nc.dram_tensor("wo", [128, EMBED], MM_DT, kind="ExternalInput").ap()
    out_d = nc.dram_tensor("out", [S, EMBED], F32, kind="ExternalOutput").ap()

    with tile.TileContext(nc) as tc:
        with (
            tc.tile_pool(name="persist", bufs=1) as persist,
            tc.tile_pool(name="pt_pool", bufs=3) as pt_pool,
            tc.tile_pool(name="norm", bufs=2) as norm_pool,
            tc.tile_pool(name="ostage", bufs=3) as ostage,
            tc.tile_pool(name="ps", bufs=2, space="PSUM") as ps_pool,
            tc.tile_pool(name="av", bufs=2, space="PSUM") as av_pool,
        ):
            # ---- weight loads on the scalar HWDGE ring (ACT idle early);
            # x chunks on the sync ring in parallel ----
            wq_sb = persist.tile([128, N_KT, 128], MM_DT)
            wk_sb = persist.tile([128, N_KT, 128], MM_DT)
            wv_sb = persist.tile([128, N_KT, 128], MM_DT)
            for w_sb, w_d in ((wk_sb, wk_d), (wq_sb, wq_d), (wv_sb, wv_d)):
                nc.scalar.dma_start(
                    out=w_sb, in_=w_d.rearrange("(t p) m -> p t m", p=128)
                )
            wo_sb = persist.tile([128, EMBED], MM_DT)
            nc.scalar.dma_start(out=wo_sb, in_=wo_d)

            xT_sb = persist.tile([128, N_KT, S], MM_DT)  # [part, ktile, seq]
            xT_r = xT_d.rearrange("(t p) s -> p t s", p=128)
            # two 2MB chunks with 4KB contiguous lines (2KB lines measured
            # only ~60-130 GB/s; bigger lines amortize per-packet overhead)
            for hb in range(2):
                hs = bass.ts(hb, 1024)
                nc.sync.dma_start(out=xT_sb[:, :, hs], in_=xT_r[:, :, hs])

            # constants; V rows padded to 72 (144B, 16B-aligned partition stride)
            V_sb = persist.tile([128, N_ST, 2, 72], BF16)
            nc.vector.memset(V_sb, 1.0)   # bakes the ones column (denominators)
            fb_sb = persist.tile([128, 1024], F32)
            nc.vector.memset(fb_sb, FE_B)
            lg_sb = persist.tile([128, 1], F32)
            nc.vector.memset(lg_sb, FE_LG)

            QT_sb = persist.tile([128, S], MM_DT)
            KT_sb = persist.tile([128, S], MM_DT)
            Z_sb = persist.tile([128, S], MM_DT)

            # ---- QKV projection emitter (phase A, folded into qt0's loop
            # so the in-order PE stream never stalls on a late x chunk) ----
            def emit_qkv(qb):
                qs = bass.ts(qb, 512)
                ps = ps_pool.tile([128, 2, 512], F32, tag="ps")
                for i, w_sb in enumerate((wk_sb, wq_sb)):
                    for kt in range(N_KT):
                        nc.tensor.matmul(
                            ps[:, i, :], w_sb[:, kt, :], xT_sb[:, kt, qs],
                            start=(kt == 0), stop=(kt == N_KT - 1),
                        )
                nc.vector.tensor_copy(KT_sb[:, qs], ps[:, 0, :])
                nc.vector.tensor_copy(QT_sb[:, qs], ps[:, 1, :])
                # V projection: two seq tiles of 128 per psum tile
                for sp_i in range(2):
                    psv = ps_pool.tile([128, 2, 512], F32, tag="ps")
                    for j in range(2):
                        st = 4 * qb + 2 * sp_i + j
                        for kt in range(N_KT):
                            nc.tensor.matmul(
                                psv[:, j, 0:128],
                                xT_sb[:, kt, bass.ts(st, 128)],
                                wv_sb[:, kt, :],
                                start=(kt == 0), stop=(kt == N_KT - 1),
                            )
                    for j in range(2):
                        st = 4 * qb + 2 * sp_i + j
                        src = psv[:, j, 0:128].rearrange("p (h d) -> p h d", h=2)
                        nc.vector.tensor_copy(V_sb[:, st, :, 0:HD], src)

            # ---- attention, software-pipelined one iteration ahead:
            # PE order [sc(p), av(p-1), sc(p+1), av(p), ...] so the ACT and
            # DVE exp instructions for consecutive iterations overlap ----
            av_tiles = []
            sc_tiles = {}
            pt_tiles = {}

            def emit_oproj_one(src_qt, mi):
                m = 4 * src_qt + mi
                po = av_tiles[src_qt][:, mi % 2, :]  # reuse freed av psum bank
                nc.tensor.matmul(
                    po, Z_sb[:, bass.ts(m, 128)], wo_sb, start=True, stop=True,
                )
                ot = ostage.tile([128, 512], F32, tag="ot")
                if mi % 2 == 0:
                    nc.scalar.copy(ot, po)
                else:
                    nc.vector.tensor_copy(ot, po)
                nc.sync.dma_start(out=out_d[bass.ts(m, 128), :], in_=ot)

            NP = N_QT * N_ST   # 64 iterations

            def emit_sc_exp(p):
                qt, ks = divmod(p, N_ST)
                qs = bass.ts(qt, 512)
                kk = bass.ts(ks, 128)
                sc = ps_pool.tile([128, 2, 512], F32, tag="ps")
                sc_tiles[p] = sc
                nc.tensor.matmul(
                    sc[:, 0, :], KT_sb[0:64, kk], QT_sb[0:64, qs],
                    start=True, stop=True, tile_position=(0, 0),
                )
                nc.tensor.matmul(
                    sc[:, 1, :], KT_sb[64:128, kk], QT_sb[64:128, qs],
                    start=True, stop=True, tile_position=(64, 0),
                )
                pt = pt_pool.tile([128, 2, 512], BF16, tag="pt")
                pt_tiles[p] = pt
                if _use_dve(qt, ks):
                    nc.vector._custom_dve(
                        EXP_FAST_ANT,
                        out=pt.bitcast(I16),
                        in0=sc,
                        in1=fb_sb,
                        s0=FE_CA,
                        s1=FE_MASK,
                        imm2=FE_C2,
                    )
                else:
                    nc.scalar.activation(
                        out=pt, in_=sc,
                        func=mybir.ActivationFunctionType.Exp,
                        scale=ACT_KAPPA, bias=lg_sb,
                    )

            def emit_av(p):
                qt, ks = divmod(p, N_ST)
                if ks == 0:
                    av_tiles.append(
                        av_pool.tile([128, 2, 512], F32, tag="av", name="av")
                    )
                av = av_tiles[qt]
                pt = pt_tiles.pop(p)
                # M=65: rows 0:64 accumulate V^T @ PT, row 64 (ones col)
                # accumulates the softmax denominators
                for h in range(2):
                    nc.tensor.matmul(
                        av[0:HD + 1, h, :], V_sb[:, ks, h, 0:HD + 1], pt[:, h, :],
                        start=(ks == 0), stop=(ks == N_ST - 1),
                    )
                if ks == N_ST - 1:
                    emit_normalize(qt)

            def emit_normalize(qt):
                qs = bass.ts(qt, 512)
                av = av_tiles[qt]
                s_row = norm_pool.tile([1, 2, 512], F32, tag="s_row")
                nc.vector.tensor_copy(s_row, av[64:65, :, :])
                r0_sb = norm_pool.tile([1, 2, 512], F32, tag="r0")
                nc.vector.reciprocal_approx_fast(out=r0_sb, in_=s_row)
                rb_sb = norm_pool.tile([64, 2, 512], F32, tag="rb")
                for h in range(2):
                    nc.gpsimd.partition_broadcast(
                        out_ap=rb_sb[0:64, h, :], in_ap=r0_sb[0:1, h, :]
                    )
                nc.vector.tensor_mul(
                    Z_sb[0:64, qs], av[0:64, 0, :], rb_sb[0:64, 0, :]
                )
                nc.vector.tensor_mul(
                    Z_sb[64:128, qs], av[0:64, 1, :], rb_sb[0:64, 1, :]
                )

            for p in range(NP + 1):
                qt, ks = divmod(p, N_ST)
                if qt == 0 and ks % 4 == 0 and p < NP:
                    emit_qkv(ks // 4)
                if qt >= 1 and ks in (4, 6, 8, 10):
                    emit_oproj_one(qt - 1, (ks - 4) // 2)
                if p < NP:
                    emit_sc_exp(p)
                if p > 0:
                    emit_av(p - 1)
            for mi in range(4):
                emit_oproj_one(N_QT - 1, mi)

    nc.compile()
    return nc


_NC = None


def _get_nc():
    global _NC
    if _NC is None:
        _NC = build_nc()
    return _NC


def make_in_maps(x, w_qkv, w_o):
    x = np.ascontiguousarray(np.asarray(x, dtype=np.float32))
    w_qkv = np.asarray(w_qkv, dtype=np.float32)
    w_o = np.asarray(w_o, dtype=np.float32)
    in_maps = []
    xTs = [np.ascontiguousarray(x[b].T) for b in range(B)]
    fe_a = np.float32(FE_A)
    for c in range(8):
        b, g = c // 4, c % 4
        cols = slice(2 * g * HD, (2 * g + 2) * HD)
        wq = np.ascontiguousarray(
            (w_qkv[:, :EMBED][:, cols] * fe_a).astype(np.float32)
        )
        in_maps.append({
            "xT": xTs[b],
            "wq": wq,
            "wk": np.ascontiguousarray(w_qkv[:, EMBED:2 * EMBED][:, cols]),
            "wv": np.ascontiguousarray(w_qkv[:, 2 * EMBED:][:, cols]),
            "wo": np.ascontiguousarray(w_o[cols, :]),
        })
    return in_maps


def combine(results, b_o):
    partials = np.stack([r["out"] for r in results])  # [8, S, EMBED]
    out = partials.reshape(B, 4, S, EMBED).sum(axis=1)
    return (out + np.asarray(b_o, dtype=np.float32)).astype(np.float32)


def kernel(x, w_qkv, w_o, b_o):
    nc = _get_nc()
    res = run_bass_kernel_spmd(nc, make_in_maps(x, w_qkv, w_o), core_ids=list(range(8)))
    return combine(res.results, b_o)
